# revision 52
# baseline (speedup 1.0000x reference)
import sys
sys.path.insert(0, '/opt/trn_rl_repo')
import math
import numpy as np
import ml_dtypes

import concourse.bass as bass
import concourse.tile as tile
from concourse import bacc, mybir
from concourse.bass_utils import run_bass_kernel_spmd
from concourse.masks import make_identity

DIM = 2048
BSZ, SEQ = 2, 2048
THRESHOLD = 0.05
S = SEQ
SB = 512
NSB = S // SB            # 4
NDC = DIM // 128         # 16
NQT = S // 128           # 16 q tiles per head
NM = 4                   # head pairs per core (head m & m+4 packed in partitions)
PTOT = 64 * NQT * (NQT + 1)   # compact causal P row length: sum 128*(qi+1)

f32 = mybir.dt.float32
f32r = mybir.dt.float32r
bf16 = mybir.dt.bfloat16
f8e4 = mybir.dt.float8e4
bf = ml_dtypes.bfloat16
f8 = ml_dtypes.float8_e4m3fn
DR = mybir.MatmulPerfMode.DoubleRow
EXP = mybir.ActivationFunctionType.Exp
AX = mybir.AxisListType.X
MAX = mybir.AluOpType.max
MIN = mybir.AluOpType.min
ADD = mybir.AluOpType.add
MUL = mybir.AluOpType.mult


def _ternarize(w):
    w = w.astype(np.float64)
    scale = max(np.abs(w).mean(), 1e-6)
    return np.where(w > THRESHOLD * scale, 1.0,
                    np.where(w < -THRESHOLD * scale, -1.0, 0.0))


def _poff(qi):
    # offset of row-block qi inside compact causal P buffer
    return 64 * qi * (qi + 1)


def build_program():
    nc = bacc.Bacc(None, target_bir_lowering=False, debug=False)

    def din(name, shape, dt):
        return nc.dram_tensor(name, list(shape), dt, kind="ExternalInput").ap()

    x_d = din("x", (DIM, S), f32r)        # x[b].T
    wq_d = din("wq", (DIM, 512), f32r)    # ternary(wq).T/8, head-pair col order
    wk_d = din("wk", (DIM, 128), f32r)    # [kv0|kv1]
    wv_d = din("wv", (DIM, 128), f32r)
    wo_d = din("wo", (128, 4, 2, DIM), f8e4)  # [part, fc, limb(w, w/16), col]
    tri_d = din("tri", (128, 128), f32)   # 0 lower/diag, -1e30 above diag
    oT_d = nc.dram_tensor("oT", [DIM, S], bf16, kind="ExternalOutput").ap()
    rr_d = nc.dram_tensor("rr", [8, NSB, SB], f32).ap()   # recip denominators

    with tile.TileContext(nc) as tc:
        with tc.tile_pool(name="persist", bufs=1) as pp:
            qt = [pp.tile([128, S], f32r, tag=f"qt{m}", name=f"qt{m}") for m in range(NM)]
            kt = pp.tile([128, S], f32r, name="kt")
            va = pp.tile([128, NDC, 130], bf16, name="va")   # per chunk: [kv0 f64|ones|kv1 f64|ones]
            ot = [pp.tile([128, S], bf16, tag=f"ot{m}", name=f"ot{m}") for m in range(NM)]
            ot8 = pp.tile([128, 4, 2, S], f8e4, name="ot8")
            tri = pp.tile([128, 128], f32, name="tri")
            nc.sync.dma_start(tri[:], tri_d[:])
            identb = pp.tile([128, 128], bf16, name="identb")
            make_identity(nc, identb[:])
            identf = pp.tile([128, 128], f32, name="identf")
            make_identity(nc, identf[:])
            nc.vector.memset(va[:, :, 64:65], 1.0)
            nc.vector.memset(va[:, :, 129:130], 1.0)

            # ---------------- phase 1: projections (fp32r) ----------------
            with tc.tile_pool(name="w1", bufs=1) as wp, \
                 tc.tile_pool(name="xp", bufs=8) as xp, \
                 tc.tile_pool(name="ev1", bufs=2) as ev, \
                 tc.tile_pool(name="ps1", bufs=1, space="PSUM") as psp:
                wq_sb = wp.tile([128, NDC, 512], f32r, name="wq_sb")
                wk_sb = wp.tile([128, NDC, 128], f32r, name="wk_sb")
                wv_sb = wp.tile([128, NDC, 128], f32r, name="wv_sb")
                for sb_i in range(NSB):
                    ssl = bass.ts(sb_i, SB)
                    ps_q = [psp.tile([128, SB], f32, tag=f"psq{m}", name=f"psq{m}")
                            for m in range(NM)]
                    ps_k = psp.tile([128, SB], f32, tag="psk")
                    ps_v = psp.tile([128, SB], f32, tag="psv")
                    for dc in range(NDC):
                        xc = xp.tile([128, SB], f32r, tag="xc")
                        nc.sync.dma_start(xc[:], x_d[dc * 128:(dc + 1) * 128, ssl])
                        if sb_i == 0 and dc == 0:
                            nc.sync.dma_start(
                                wq_sb[:], wq_d.rearrange("(a p) b -> p a b", p=128))
                            nc.sync.dma_start(
                                wk_sb[:], wk_d.rearrange("(a p) b -> p a b", p=128))
                            nc.sync.dma_start(
                                wv_sb[:], wv_d.rearrange("(a p) b -> p a b", p=128))
                        st = (dc == 0)
                        sp = (dc == NDC - 1)
                        for m in range(NM):
                            nc.tensor.matmul(ps_q[m][:], wq_sb[:, dc, bass.ts(m, 128)],
                                             xc[:], start=st, stop=sp)
                        nc.tensor.matmul(ps_k[:], wk_sb[:, dc, :], xc[:], start=st, stop=sp)
                        nc.tensor.matmul(ps_v[:], wv_sb[:, dc, :], xc[:], start=st, stop=sp)
                    # evacuate
                    for m in range(NM):
                        cp1 = nc.scalar.copy if m % 2 else nc.vector.tensor_copy
                        cp1(qt[m][:, ssl], ps_q[m][:])
                    nc.scalar.copy(kt[:, ssl], ps_k[:])
                    vts = ev.tile([128, SB], f32, tag="vts")
                    nc.vector.tensor_copy(vts[:], ps_v[:])
                    # transpose V^T[f, s-chunk] -> V[s, f] per 128-block
                    ps_t = psp.tile([128, 4, 128], f32, tag="pst")
                    for j in range(4):
                        nc.tensor.matmul(ps_t[:, j, :], vts[:, bass.ts(j, 128)],
                                         identf[:], is_transpose=True,
                                         start=True, stop=True)
                    for j in range(4):
                        ch = sb_i * 4 + j
                        nc.vector.tensor_copy(va[:, ch, 0:64], ps_t[:, j, 0:64])
                        nc.vector.tensor_copy(va[:, ch, 65:129], ps_t[:, j, 64:128])

            # ---------------- phase 2: attention ----------------
            with tc.tile_pool(name="att", bufs=1) as ap, \
                 tc.tile_pool(name="stat", bufs=3) as stp, \
                 tc.tile_pool(name="nrm", bufs=1) as nrm, \
                 tc.tile_pool(name="ps2", bufs=1, space="PSUM") as ps2:
                p_t = [ap.tile([128, PTOT], bf16, tag=f"p{i}", name=f"p{i}") for i in range(2)]
                pt_t = [ap.tile([128, NDC, SB], bf16, tag=f"pt{i}", name=f"pt{i}")
                        for i in range(2)]

                # tp-work queue: list of closures from previous stage
                pending = []

                def emit_some(k):
                    for _ in range(min(k, len(pending))):
                        pending.pop(0)()

                def make_tp_work(m, h, pbuf):
                    """DMA-transpose + PV closures for stage (m, h); the
                    1/denominator normalize is deferred to a per-stage batch."""
                    units = []
                    hg = m + 4 * h

                    def tp_chunk(qb, c):
                        def run():
                            ptb = pt_t[qb % 2]
                            jlo = max(0, c - 4 * qb)
                            tps = ps2.tile([128, SB], bf16, tag="tps", bufs=1,
                                           name="tps")
                            for j in range(jlo, 4):
                                qj = 4 * qb + j
                                nc.tensor.matmul(
                                    tps[:, bass.ts(j, 128)],
                                    pbuf[:, bass.ds(_poff(qj) + 128 * c, 128)],
                                    identb[:], is_transpose=True,
                                    start=(j == jlo), stop=(j == 3))
                            if jlo > 0:
                                nc.gpsimd.memset(ptb[:, c, 0:jlo * 128], 0.0)
                            cp = nc.vector.tensor_copy if c % 2 else nc.scalar.copy
                            cp(ptb[:, c, bass.ds(jlo * 128, (4 - jlo) * 128)],
                               tps[:, bass.ds(jlo * 128, (4 - jlo) * 128)])
                        return run

                    def pv_qb(qb):
                        def run():
                            nch = 4 * (qb + 1)
                            ptb = pt_t[qb % 2]
                            pv = ps2.tile([65, SB], f32, tag="pv", name="pv")
                            for c in range(nch):
                                nc.tensor.matmul(pv[:], va[:, c, bass.ds(65 * h, 65)],
                                                 ptb[:, c, :],
                                                 start=(c == 0), stop=(c == nch - 1))
                            rr = stp.tile([1, SB], f32, tag="rr", name="rr")
                            nc.vector.reciprocal(rr[:], pv[64:65, :])
                            nc.sync.dma_start(rr_d[hg, qb, :], rr[:])
                            if qb % 2 == 0:
                                nc.scalar.copy(
                                    ot[m][bass.ds(64 * h, 64), bass.ts(qb, SB)],
                                    pv[0:64, :])
                            else:
                                nc.vector.tensor_copy(
                                    ot[m][bass.ds(64 * h, 64), bass.ts(qb, SB)],
                                    pv[0:64, :])
                        return run

                    def norm_stage():
                        def run():
                            hsl = bass.ds(64 * h, 64)
                            rrb = nrm.tile([128, S], f32, tag="rrb", name="rrb")
                            nc.sync.dma_start(
                                rrb[hsl, :],
                                rr_d[hg:hg + 1, :, :].rearrange("a b c -> a (b c)")
                                .to_broadcast((64, S)))
                            # normalize + split into two fp8 limbs (on Pool)
                            nc.gpsimd.tensor_mul(ot[m][hsl, :], ot[m][hsl, :],
                                                 rrb[hsl, :])
                            nc.gpsimd.tensor_copy(ot8[hsl, m, 0, :], ot[m][hsl, :])
                            nc.gpsimd.tensor_sub(ot8[hsl, m, 1, :], ot[m][hsl, :],
                                                 ot8[hsl, m, 0, :])
                        return run

                    for qb in range(NSB):
                        for c in range(4 * (qb + 1)):
                            units.append(tp_chunk(qb, c))
                        units.append(pv_qb(qb))
                    units.append(norm_stage())
                    return units

                for stage in range(8):
                    m, h = stage % 4, stage // 4
                    hs = bass.ds(64 * h, 64)
                    pbuf = p_t[stage % 2]
                    nmx_p = [None] * NQT   # negmax [128,1] per row
                    for qi in range(NQT):
                        # ---- pass 1: row maxes; diag chunk exp'd in place ----
                        kw = 128 * (qi + 1)
                        nk = (kw + 511) // 512
                        nmx = stp.tile([128, 4], f32, tag="nmx", name="nmx")
                        s1_last = None
                        nhalf = (nk + 1) // 2
                        for hf in range(nhalf):
                            ck = min(2, nk - 2 * hf)         # chunks in this tile
                            w = min(1024, kw - 1024 * hf)
                            s1 = ps2.tile([128, 2, SB], f32, tag="s1", bufs=2,
                                          name="s1")
                            for c2 in range(ck):
                                cw = min(512, w - 512 * c2)
                                nc.tensor.matmul(
                                    s1[:, c2, 0:cw],
                                    qt[m][hs, bass.ts(qi, 128)],
                                    kt[hs, bass.ds(1024 * hf + 512 * c2, cw)],
                                    start=True, stop=True)
                            if hf == nhalf - 1:
                                cl, cwl = ck - 1, w - 512 * (ck - 1)
                                nc.vector.tensor_add(
                                    s1[:, cl, cwl - 128:cwl],
                                    s1[:, cl, cwl - 128:cwl], tri[:])
                                s1_last = (s1, cl, cwl)
                            # one reduce covers both chunks when full
                            if w == 1024:
                                nc.vector.tensor_reduce(
                                    nmx[:, 2 * hf:2 * hf + 2], s1[:, :, :],
                                    AX, MAX, negate=True)
                            else:
                                nc.vector.tensor_reduce(
                                    nmx[:, 2 * hf:2 * hf + 1],
                                    s1[:, 0, 0:w] if ck == 1 else s1[:, 1, 0:w - 512],
                                    AX, MAX, negate=True)
                                if ck == 2:
                                    nc.vector.tensor_reduce(
                                        nmx[:, 2 * hf + 1:2 * hf + 2],
                                        s1[:, 0, 0:512], AX, MAX, negate=True)
                        ngm = stp.tile([128, 1], f32, tag="ngm", name="ngm")
                        nc.vector.tensor_reduce(ngm[:], nmx[:, 0:nk], AX, MIN)
                        nmx_p[qi] = ngm
                        # diagonal (last) chunk: exp directly from pass-1 psum
                        s1, cl, cwl = s1_last
                        nc.scalar.activation(
                            pbuf[:, bass.ds(_poff(qi) + 512 * (nk - 1), cwl)],
                            s1[:, cl, 0:cwl], EXP, bias=ngm[:], scale=1.0)
                        if nk == 2:
                            # whole row lives in this psum tile: exp chunk 0 too
                            nc.scalar.activation(
                                pbuf[:, bass.ds(_poff(qi), 512)],
                                s1[:, 0, :], EXP, bias=ngm[:], scale=1.0)
                        # ---- pass 2 (non-diagonal chunks) for previous row ----
                        if qi >= 1:
                            emit_pass2(nc, ps2, qt, kt, p_t, nmx_p, m, h, qi - 1,
                                       stage)
                        emit_some(3)
                    emit_pass2(nc, ps2, qt, kt, p_t, nmx_p, m, h, NQT - 1, stage)
                    emit_some(7)
                    pending.extend(make_tp_work(m, h, pbuf))
                    if stage == 7:
                        emit_some(10 ** 6)

            # ---------------- phase 3: output projection ----------------
            with tc.tile_pool(name="wop", bufs=1) as wp3, \
                 tc.tile_pool(name="op", bufs=4) as op, \
                 tc.tile_pool(name="ps3", bufs=3, space="PSUM") as ps3:
                wo_sb = wp3.tile([128, 4, 2, DIM], f8e4, name="wo_sb")
                nc.sync.dma_start(wo_sb[:], wo_d[:])
                for mo in range(16):
                    for sb_i in range(NSB):
                        ps_o = ps3.tile([128, SB], f32, tag="pso")
                        for fc in range(4):
                            nc.tensor.matmul(
                                ps_o[:], wo_sb[:, fc, :, bass.ts(mo, 128)],
                                ot8[:, fc, :, bass.ts(sb_i, SB)],
                                start=(fc == 0), stop=(fc == 3),
                                perf_mode=DR)
                        osb = op.tile([128, SB], bf16, tag="osb")
                        if (mo * NSB + sb_i) % 2 == 0:
                            nc.scalar.copy(osb[:], ps_o[:])
                        else:
                            nc.vector.tensor_copy(osb[:], ps_o[:])
                        nc.scalar.dma_start(
                            oT_d[bass.ts(mo, 128), bass.ts(sb_i, SB)], osb[:])

    nc.compile()
    return nc


def emit_pass2(nc, ps2, qt, kt, p_t, nmx_p, m, h, qi, stage):
    """Recompute the full 512-wide (non-diagonal) score chunks for row qi
    and exp them into p. The diagonal chunk was exp'd from pass-1 psum."""
    hs = bass.ds(64 * h, 64)
    pbuf = p_t[stage % 2]
    kw = 128 * (qi + 1)
    nfull = (kw - 1) // 512          # chunks before the diagonal one
    if nfull <= 1 and kw <= 1024:    # row fully exp'd from pass-1 psum
        return
    if nfull == 0:
        return
    ngm = nmx_p[qi]
    for half in range(0, 512 * nfull, 1024):
        hw_ = min(1024, 512 * nfull - half)
        s2 = ps2.tile([128, 1024], f32, tag="s2", bufs=1, name="s2")
        for c0 in range(0, hw_, 512):
            nc.tensor.matmul(s2[:, c0:c0 + 512],
                             qt[m][hs, bass.ts(qi, 128)],
                             kt[hs, bass.ds(half + c0, 512)],
                             start=True, stop=True)
        nc.scalar.activation(pbuf[:, bass.ds(_poff(qi) + half, hw_)],
                             s2[:, 0:hw_], EXP, bias=ngm[:], scale=1.0)


_PROG = None


def kernel(x, wq, wk, wv, wo):
    global _PROG
    if _PROG is None:
        _PROG = build_program()
    nc = _PROG

    twq = _ternarize(wq) / 8.0          # fold softmax scale into q
    twk = _ternarize(wk)
    twv = _ternarize(wv)
    two = _ternarize(wo)
    tri_np = ((1.0 - np.tril(np.ones((128, 128)))) * -1e30).astype(np.float32)

    # head-pair permutation: pair m holds heads (m, m+4) of the local group
    perm = []
    for m in range(4):
        perm += list(range(64 * m, 64 * m + 64))
        perm += list(range(64 * (m + 4), 64 * (m + 4) + 64))

    in_maps = []
    for c in range(8):
        b, hq = c % 2, c // 2
        xT = np.ascontiguousarray(x[b].astype(np.float32).T)      # [DIM, S]
        gq = slice(hq * 512, (hq + 1) * 512)
        gkv = slice(hq * 128, (hq + 1) * 128)
        wq_cols = twq.T[:, gq][:, perm]                            # [DIM, 512]
        wo_rows = two.T[gq, :][perm, :]                            # [512, DIM]
        wo8 = np.zeros((128, 4, 2, DIM), np.float32)
        for fc in range(4):
            wo8[:, fc, 0, :] = wo_rows[fc * 128:(fc + 1) * 128]
            wo8[:, fc, 1, :] = wo_rows[fc * 128:(fc + 1) * 128]
        in_maps.append({
            "x": xT,
            "wq": np.ascontiguousarray(wq_cols).astype(np.float32),
            "wk": np.ascontiguousarray(twk.T[:, gkv]).astype(np.float32),
            "wv": np.ascontiguousarray(twv.T[:, gkv]).astype(np.float32),
            "wo": wo8.astype(f8),
            "tri": tri_np,
        })

    res = run_bass_kernel_spmd(nc, in_maps, list(range(8)))

    out = np.zeros((BSZ, SEQ, DIM), np.float32)
    for c in range(8):
        b = c % 2
        out[b] += res.results[c]["oT"].astype(np.float32).T
    return out


# revision 53
# speedup vs baseline: 1.0048x; 1.0048x over previous
import sys
sys.path.insert(0, '/opt/trn_rl_repo')
import math
import numpy as np
import ml_dtypes

import concourse.bass as bass
import concourse.tile as tile
from concourse import bacc, mybir
from concourse.bass_utils import run_bass_kernel_spmd
from concourse.masks import make_identity

DIM = 2048
BSZ, SEQ = 2, 2048
THRESHOLD = 0.05
S = SEQ
SB = 512
NSB = S // SB            # 4
NDC = DIM // 128         # 16
NQT = S // 128           # 16 q tiles per head
NM = 4                   # head pairs per core (head m & m+4 packed in partitions)
PTOT = 64 * NQT * (NQT + 1)   # compact causal P row length: sum 128*(qi+1)

f32 = mybir.dt.float32
f32r = mybir.dt.float32r
bf16 = mybir.dt.bfloat16
f8e4 = mybir.dt.float8e4
bf = ml_dtypes.bfloat16
f8 = ml_dtypes.float8_e4m3fn
DR = mybir.MatmulPerfMode.DoubleRow
EXP = mybir.ActivationFunctionType.Exp
AX = mybir.AxisListType.X
MAX = mybir.AluOpType.max
MIN = mybir.AluOpType.min
ADD = mybir.AluOpType.add
MUL = mybir.AluOpType.mult


def _ternarize(w):
    w = w.astype(np.float64)
    scale = max(np.abs(w).mean(), 1e-6)
    return np.where(w > THRESHOLD * scale, 1.0,
                    np.where(w < -THRESHOLD * scale, -1.0, 0.0))


def _poff(qi):
    # offset of row-block qi inside compact causal P buffer
    return 64 * qi * (qi + 1)


def build_program():
    nc = bacc.Bacc(None, target_bir_lowering=False, debug=False)

    def din(name, shape, dt):
        return nc.dram_tensor(name, list(shape), dt, kind="ExternalInput").ap()

    x_d = din("x", (DIM, S), f32r)        # x[b].T
    wq_d = din("wq", (DIM, 512), f32r)    # ternary(wq).T/8, head-pair col order
    wk_d = din("wk", (DIM, 128), f32r)    # [kv0|kv1]
    wv_d = din("wv", (DIM, 128), f32r)
    wo_d = din("wo", (128, 4, 2, DIM), f8e4)  # [part, fc, limb(w, w/16), col]
    tri_d = din("tri", (128, 128), f32)   # 0 lower/diag, -1e30 above diag
    oT_d = nc.dram_tensor("oT", [DIM, S], bf16, kind="ExternalOutput").ap()
    rr_d = nc.dram_tensor("rr", [8, NSB, SB], f32).ap()   # recip denominators

    with tile.TileContext(nc) as tc:
        with tc.tile_pool(name="persist", bufs=1) as pp:
            qt = [pp.tile([128, S], f32r, tag=f"qt{m}", name=f"qt{m}") for m in range(NM)]
            kt = pp.tile([128, S], f32r, name="kt")
            va = pp.tile([128, NDC, 130], bf16, name="va")   # per chunk: [kv0 f64|ones|kv1 f64|ones]
            ot = [pp.tile([128, S], bf16, tag=f"ot{m}", name=f"ot{m}") for m in range(NM)]
            ot8 = pp.tile([128, 4, 2, S], f8e4, name="ot8")
            tri = pp.tile([128, 128], f32, name="tri")
            nc.sync.dma_start(tri[:], tri_d[:])
            identb = pp.tile([128, 128], bf16, name="identb")
            make_identity(nc, identb[:])
            identf = pp.tile([128, 128], f32, name="identf")
            make_identity(nc, identf[:])
            nc.vector.memset(va[:, :, 64:65], 1.0)
            nc.vector.memset(va[:, :, 129:130], 1.0)

            # ---------------- phase 1: projections (fp32r) ----------------
            with tc.tile_pool(name="w1", bufs=1) as wp, \
                 tc.tile_pool(name="xp", bufs=8) as xp, \
                 tc.tile_pool(name="ev1", bufs=2) as ev, \
                 tc.tile_pool(name="ps1", bufs=1, space="PSUM") as psp:
                wq_sb = wp.tile([128, NDC, 512], f32r, name="wq_sb")
                wk_sb = wp.tile([128, NDC, 128], f32r, name="wk_sb")
                wv_sb = wp.tile([128, NDC, 128], f32r, name="wv_sb")
                for sb_i in range(NSB):
                    ssl = bass.ts(sb_i, SB)
                    ps_q = [psp.tile([128, SB], f32, tag=f"psq{m}", name=f"psq{m}")
                            for m in range(NM)]
                    ps_k = psp.tile([128, SB], f32, tag="psk")
                    ps_v = psp.tile([128, SB], f32, tag="psv")
                    for dc in range(NDC):
                        xc = xp.tile([128, SB], f32r, tag="xc")
                        nc.sync.dma_start(xc[:], x_d[dc * 128:(dc + 1) * 128, ssl])
                        if sb_i == 0 and dc == 0:
                            nc.sync.dma_start(
                                wq_sb[:], wq_d.rearrange("(a p) b -> p a b", p=128))
                            nc.sync.dma_start(
                                wk_sb[:], wk_d.rearrange("(a p) b -> p a b", p=128))
                            nc.sync.dma_start(
                                wv_sb[:], wv_d.rearrange("(a p) b -> p a b", p=128))
                        st = (dc == 0)
                        sp = (dc == NDC - 1)
                        for m in range(NM):
                            nc.tensor.matmul(ps_q[m][:], wq_sb[:, dc, bass.ts(m, 128)],
                                             xc[:], start=st, stop=sp)
                        nc.tensor.matmul(ps_k[:], wk_sb[:, dc, :], xc[:], start=st, stop=sp)
                        nc.tensor.matmul(ps_v[:], wv_sb[:, dc, :], xc[:], start=st, stop=sp)
                    # evacuate
                    for m in range(NM):
                        cp1 = nc.scalar.copy if m % 2 else nc.vector.tensor_copy
                        cp1(qt[m][:, ssl], ps_q[m][:])
                    nc.scalar.copy(kt[:, ssl], ps_k[:])
                    vts = ev.tile([128, SB], f32, tag="vts")
                    nc.vector.tensor_copy(vts[:], ps_v[:])
                    # transpose V^T[f, s-chunk] -> V[s, f] per 128-block
                    ps_t = psp.tile([128, 4, 128], f32, tag="pst")
                    for j in range(4):
                        nc.tensor.matmul(ps_t[:, j, :], vts[:, bass.ts(j, 128)],
                                         identf[:], is_transpose=True,
                                         start=True, stop=True)
                    for j in range(4):
                        ch = sb_i * 4 + j
                        nc.vector.tensor_copy(va[:, ch, 0:64], ps_t[:, j, 0:64])
                        nc.vector.tensor_copy(va[:, ch, 65:129], ps_t[:, j, 64:128])

            # ---------------- phase 2: attention ----------------
            with tc.tile_pool(name="att", bufs=1) as ap, \
                 tc.tile_pool(name="stat", bufs=3) as stp, \
                 tc.tile_pool(name="nrm", bufs=1) as nrm, \
                 tc.tile_pool(name="ps2", bufs=1, space="PSUM") as ps2:
                p_t = [ap.tile([128, PTOT], bf16, tag=f"p{i}", name=f"p{i}") for i in range(2)]
                pt_t = [ap.tile([128, NDC, SB], bf16, tag=f"pt{i}", name=f"pt{i}")
                        for i in range(2)]

                # tp-work queue: list of closures from previous stage
                pending = []

                def emit_some(k):
                    for _ in range(min(k, len(pending))):
                        pending.pop(0)()

                def make_tp_work(m, h, pbuf):
                    """DMA-transpose + PV closures for stage (m, h); the
                    1/denominator normalize is deferred to a per-stage batch."""
                    units = []
                    hg = m + 4 * h

                    def tp_chunk(qb, c):
                        def run():
                            ptb = pt_t[qb % 2]
                            jlo = max(0, c - 4 * qb)
                            tps = ps2.tile([128, SB], bf16, tag="tps", bufs=1,
                                           name="tps")
                            for j in range(jlo, 4):
                                qj = 4 * qb + j
                                nc.tensor.matmul(
                                    tps[:, bass.ts(j, 128)],
                                    pbuf[:, bass.ds(_poff(qj) + 128 * c, 128)],
                                    identb[:], is_transpose=True,
                                    start=(j == jlo), stop=(j == 3))
                            if jlo > 0:
                                nc.gpsimd.memset(ptb[:, c, 0:jlo * 128], 0.0)
                            cp = nc.vector.tensor_copy if c % 2 else nc.scalar.copy
                            cp(ptb[:, c, bass.ds(jlo * 128, (4 - jlo) * 128)],
                               tps[:, bass.ds(jlo * 128, (4 - jlo) * 128)])
                        return run

                    def pv_qb(qb):
                        def run():
                            nch = 4 * (qb + 1)
                            ptb = pt_t[qb % 2]
                            pv = ps2.tile([65, SB], f32, tag="pv", name="pv")
                            for c in range(nch):
                                nc.tensor.matmul(pv[:], va[:, c, bass.ds(65 * h, 65)],
                                                 ptb[:, c, :],
                                                 start=(c == 0), stop=(c == nch - 1))
                            rr = stp.tile([1, SB], f32, tag="rr", name="rr")
                            nc.vector.reciprocal(rr[:], pv[64:65, :])
                            nc.sync.dma_start(rr_d[hg, qb, :], rr[:])
                            if qb % 2 == 0:
                                nc.scalar.copy(
                                    ot[m][bass.ds(64 * h, 64), bass.ts(qb, SB)],
                                    pv[0:64, :])
                            else:
                                nc.vector.tensor_copy(
                                    ot[m][bass.ds(64 * h, 64), bass.ts(qb, SB)],
                                    pv[0:64, :])
                        return run

                    def norm_stage():
                        def run():
                            hsl = bass.ds(64 * h, 64)
                            rrb = nrm.tile([128, S], f32, tag="rrb", name="rrb")
                            nc.sync.dma_start(
                                rrb[hsl, :],
                                rr_d[hg:hg + 1, :, :].rearrange("a b c -> a (b c)")
                                .to_broadcast((64, S)))
                            # normalize + split into two fp8 limbs (on Pool)
                            nc.gpsimd.tensor_mul(ot[m][hsl, :], ot[m][hsl, :],
                                                 rrb[hsl, :])
                            nc.gpsimd.tensor_copy(ot8[hsl, m, 0, :], ot[m][hsl, :])
                            nc.gpsimd.tensor_sub(ot8[hsl, m, 1, :], ot[m][hsl, :],
                                                 ot8[hsl, m, 0, :])
                        return run

                    for qb in range(NSB):
                        for c in range(4 * (qb + 1)):
                            units.append(tp_chunk(qb, c))
                        units.append(pv_qb(qb))
                    units.append(norm_stage())
                    return units

                for stage in range(8):
                    m, h = stage % 4, stage // 4
                    hs = bass.ds(64 * h, 64)
                    pbuf = p_t[stage % 2]
                    nmx_p = [None] * NQT   # negmax [128,1] per row
                    for qi in range(NQT):
                        # ---- pass 1: row maxes; diag chunk exp'd in place ----
                        kw = 128 * (qi + 1)
                        nk = (kw + 511) // 512
                        nmx = stp.tile([128, 4], f32, tag="nmx", name="nmx")
                        s1_last = None
                        nhalf = (nk + 1) // 2
                        for hf in range(nhalf):
                            ck = min(2, nk - 2 * hf)         # chunks in this tile
                            w = min(1024, kw - 1024 * hf)
                            s1 = ps2.tile([128, 2, SB], f32, tag="s1", bufs=2,
                                          name="s1")
                            for c2 in range(ck):
                                cw = min(512, w - 512 * c2)
                                nc.tensor.matmul(
                                    s1[:, c2, 0:cw],
                                    qt[m][hs, bass.ts(qi, 128)],
                                    kt[hs, bass.ds(1024 * hf + 512 * c2, cw)],
                                    start=True, stop=True)
                            if hf == nhalf - 1:
                                cl, cwl = ck - 1, w - 512 * (ck - 1)
                                nc.vector.tensor_add(
                                    s1[:, cl, cwl - 128:cwl],
                                    s1[:, cl, cwl - 128:cwl], tri[:])
                                s1_last = (s1, cl, cwl)
                            # one reduce covers both chunks when full
                            if w == 1024:
                                nc.vector.tensor_reduce(
                                    nmx[:, 2 * hf:2 * hf + 2], s1[:, :, :],
                                    AX, MAX, negate=True)
                            else:
                                nc.vector.tensor_reduce(
                                    nmx[:, 2 * hf:2 * hf + 1],
                                    s1[:, 0, 0:w] if ck == 1 else s1[:, 1, 0:w - 512],
                                    AX, MAX, negate=True)
                                if ck == 2:
                                    nc.vector.tensor_reduce(
                                        nmx[:, 2 * hf + 1:2 * hf + 2],
                                        s1[:, 0, 0:512], AX, MAX, negate=True)
                        ngm = stp.tile([128, 1], f32, tag="ngm", name="ngm")
                        nc.vector.tensor_reduce(ngm[:], nmx[:, 0:nk], AX, MIN)
                        nmx_p[qi] = ngm
                        # diagonal (last) chunk: exp directly from pass-1 psum
                        s1, cl, cwl = s1_last
                        nc.scalar.activation(
                            pbuf[:, bass.ds(_poff(qi) + 512 * (nk - 1), cwl)],
                            s1[:, cl, 0:cwl], EXP, bias=ngm[:], scale=1.0)
                        if nk == 2:
                            # whole row lives in this psum tile: exp chunk 0 too
                            nc.scalar.activation(
                                pbuf[:, bass.ds(_poff(qi), 512)],
                                s1[:, 0, :], EXP, bias=ngm[:], scale=1.0)
                        # ---- pass 2 (non-diagonal chunks) for previous row ----
                        if qi >= 1:
                            emit_pass2(nc, ps2, qt, kt, p_t, nmx_p, m, h, qi - 1,
                                       stage)
                        emit_some(3)
                    emit_pass2(nc, ps2, qt, kt, p_t, nmx_p, m, h, NQT - 1, stage)
                    emit_some(7)
                    pending.extend(make_tp_work(m, h, pbuf))
                    if stage == 7:
                        emit_some(10 ** 6)

            # ---------------- phase 3: output projection ----------------
            with tc.tile_pool(name="wop", bufs=1) as wp3, \
                 tc.tile_pool(name="op", bufs=8) as op, \
                 tc.tile_pool(name="ps3", bufs=6, space="PSUM") as ps3:
                wo_sb = wp3.tile([128, 4, 2, DIM], f8e4, name="wo_sb")
                nc.sync.dma_start(wo_sb[:], wo_d[:])
                for mo in range(16):
                    for sb_i in range(NSB):
                        ps_o = ps3.tile([128, SB], f32, tag="pso")
                        for fc in range(4):
                            nc.tensor.matmul(
                                ps_o[:], wo_sb[:, fc, :, bass.ts(mo, 128)],
                                ot8[:, fc, :, bass.ts(sb_i, SB)],
                                start=(fc == 0), stop=(fc == 3),
                                perf_mode=DR)
                        osb = op.tile([128, SB], bf16, tag="osb")
                        if (mo * NSB + sb_i) % 2 == 0:
                            nc.scalar.copy(osb[:], ps_o[:])
                        else:
                            nc.vector.tensor_copy(osb[:], ps_o[:])
                        nc.scalar.dma_start(
                            oT_d[bass.ts(mo, 128), bass.ts(sb_i, SB)], osb[:])

    nc.compile()
    return nc


def emit_pass2(nc, ps2, qt, kt, p_t, nmx_p, m, h, qi, stage):
    """Recompute the full 512-wide (non-diagonal) score chunks for row qi
    and exp them into p. The diagonal chunk was exp'd from pass-1 psum."""
    hs = bass.ds(64 * h, 64)
    pbuf = p_t[stage % 2]
    kw = 128 * (qi + 1)
    nfull = (kw - 1) // 512          # chunks before the diagonal one
    if nfull <= 1 and kw <= 1024:    # row fully exp'd from pass-1 psum
        return
    if nfull == 0:
        return
    ngm = nmx_p[qi]
    for half in range(0, 512 * nfull, 1024):
        hw_ = min(1024, 512 * nfull - half)
        s2 = ps2.tile([128, 1024], f32, tag="s2", bufs=1, name="s2")
        for c0 in range(0, hw_, 512):
            nc.tensor.matmul(s2[:, c0:c0 + 512],
                             qt[m][hs, bass.ts(qi, 128)],
                             kt[hs, bass.ds(half + c0, 512)],
                             start=True, stop=True)
        nc.scalar.activation(pbuf[:, bass.ds(_poff(qi) + half, hw_)],
                             s2[:, 0:hw_], EXP, bias=ngm[:], scale=1.0)


_PROG = None


def kernel(x, wq, wk, wv, wo):
    global _PROG
    if _PROG is None:
        _PROG = build_program()
    nc = _PROG

    twq = _ternarize(wq) / 8.0          # fold softmax scale into q
    twk = _ternarize(wk)
    twv = _ternarize(wv)
    two = _ternarize(wo)
    tri_np = ((1.0 - np.tril(np.ones((128, 128)))) * -1e30).astype(np.float32)

    # head-pair permutation: pair m holds heads (m, m+4) of the local group
    perm = []
    for m in range(4):
        perm += list(range(64 * m, 64 * m + 64))
        perm += list(range(64 * (m + 4), 64 * (m + 4) + 64))

    in_maps = []
    for c in range(8):
        b, hq = c % 2, c // 2
        xT = np.ascontiguousarray(x[b].astype(np.float32).T)      # [DIM, S]
        gq = slice(hq * 512, (hq + 1) * 512)
        gkv = slice(hq * 128, (hq + 1) * 128)
        wq_cols = twq.T[:, gq][:, perm]                            # [DIM, 512]
        wo_rows = two.T[gq, :][perm, :]                            # [512, DIM]
        wo8 = np.zeros((128, 4, 2, DIM), np.float32)
        for fc in range(4):
            wo8[:, fc, 0, :] = wo_rows[fc * 128:(fc + 1) * 128]
            wo8[:, fc, 1, :] = wo_rows[fc * 128:(fc + 1) * 128]
        in_maps.append({
            "x": xT,
            "wq": np.ascontiguousarray(wq_cols).astype(np.float32),
            "wk": np.ascontiguousarray(twk.T[:, gkv]).astype(np.float32),
            "wv": np.ascontiguousarray(twv.T[:, gkv]).astype(np.float32),
            "wo": wo8.astype(f8),
            "tri": tri_np,
        })

    res = run_bass_kernel_spmd(nc, in_maps, list(range(8)))

    out = np.zeros((BSZ, SEQ, DIM), np.float32)
    for c in range(8):
        b = c % 2
        out[b] += res.results[c]["oT"].astype(np.float32).T
    return out
